# revision 1
# baseline (speedup 1.0000x reference)
"""Trainium2 Bass kernel for nn_AttentionHead (sparse attention via per-timestep
ISTA with spectral step size).

Per batch element (data-parallel over 8 NeuronCores):
  Q/K/V projections; Qs = Q/sqrt(dk), Ks = K/sqrt(dk).
  lam_max(t) = top eigenvalue of the cumulative Gram G_t = sum_{s<=t} k_s k_s^T,
  computed with dual-candidate Chebyshev-accelerated power iteration (replaces
  eigvalsh; the output is insensitive to ~1% lambda error).
  eta_t = 0.9/(lam_max + 1e-8).
  ISTA runs in "beta space" (alpha = eta*beta), making the soft threshold the
  CONSTANT lambda=SPARSITY:
      beta <- mask * softthr_0.05( beta + Ks @ (QsT - eta*(Ks^T beta)) )
  out[t] = eta_t * (beta^T V)[t].

Matmuls run as float32r (full-rate reduced fp32, ~1.5e-4 rel err measured on
HW); the power phase is bf16. Causal structure is exploited: only the upper
triangle (s <= t) of every [S,S] object is computed.
"""
import numpy as np

B, S, DM, DK = 8, 1024, 512, 64
NUM_ITER, SPARSITY = 40, 0.05
N_CORES = 8
P = 128
NT = S // P   # 8 s-tiles

CFG = dict(
    const_eta=None,   # debug: float -> skip power phase, use constant eta
    n_pre=3,        # plain power apps before Chebyshev (RQ -> fixed b)
    n_cheb=8,       # Chebyshev-accelerated apps
    num_iter=NUM_ITER,
    # threshold scheme per s-tile: 'act' (2 ACT relus + TT combine) or
    # 'dve' (DVE clamp + TT).  combine engine per tile: 'v'(DVE) or 'g'(GPSIMD)
    thr_scheme=['act'] * 8,
    combine_eng=['g', 'v', 'g', 'v', 'g', 'v', 'g', 'v'],
    mask_eng=['v', 'g', 'v', 'g', 'v', 'g', 'v', 'g'],
    ccopy_eng=['v', 's', 'v', 's', 'v', 's', 'v', 's'],  # power C copy split
)

_RUNNER = None


def _chunks(c0, end=S, step=512):
    """512-aligned column chunks covering [c0, end)."""
    out = []
    c = c0
    while c < end:
        nxt = min(end, (c // step + 1) * step)
        out.append(slice(c, nxt))
        c = nxt
    return out


# ---------------------------------------------------------------------------
# wait-spill: this container's walrus allows ONE sem-wait per instruction.
# Move extras onto same-engine NoOps placed immediately before the offender.
# ---------------------------------------------------------------------------
def _spill_excess_waits(nc, max_waits=1):
    from concourse import mybir
    k = [0]
    for f in nc.m.functions:
        for bb in f.blocks:
            insts = list(bb.instructions)
            out_l, ch = [], False
            for ins_ in insts:
                si = ins_.sync_info
                w = list(si.on_wait) if si else []
                if len(w) > max_waits:
                    ch = True
                    si.on_wait = w[:max_waits]
                    for j in range(max_waits, len(w), max_waits):
                        k[0] += 1
                        nop = mybir.InstNoOp(name=f"ws-{k[0]}")
                        nop.engine = ins_.engine
                        nop.sync_info = type(si)(on_wait=w[j:j + max_waits],
                                                 on_update=[])
                        out_l.append(nop)
                out_l.append(ins_)
            if ch:
                bb.instructions = out_l
    return nc


# ---------------------------------------------------------------------------
# Bass program (one core = one batch element)
# ---------------------------------------------------------------------------
def _build_nc(cfg):
    import concourse.bass as bass
    import concourse.tile as tile
    from concourse import mybir

    f32 = mybir.dt.float32
    f32r = mybir.dt.float32r
    bf16 = mybir.dt.bfloat16
    AF = mybir.ActivationFunctionType
    OP = mybir.AluOpType

    nc = bass.Bass()

    x = nc.declare_dram_parameter("x", [S, DM], f32, isOutput=False)
    wq = nc.declare_dram_parameter("wq", [DM, DK], f32, isOutput=False)
    wk = nc.declare_dram_parameter("wk", [DM, DK], f32, isOutput=False)
    wv = nc.declare_dram_parameter("wv", [DM, DK], f32, isOutput=False)
    bq = nc.declare_dram_parameter("bq", [DK], f32, isOutput=False)
    bk = nc.declare_dram_parameter("bk", [DK], f32, isOutput=False)
    bv = nc.declare_dram_parameter("bv", [DK], f32, isOutput=False)
    rinit = nc.declare_dram_parameter("rinit", [P, S], f32, isOutput=False)
    trim = nc.declare_dram_parameter("trim", [P, P], f32, isOutput=False)
    ident = nc.declare_dram_parameter("ident", [P, P], f32, isOutput=False)
    ccol = nc.declare_dram_parameter("ccol", [P, 4], f32, isOutput=False)
    crow = nc.declare_dram_parameter("crow", [33, S], f32, isOutput=False)
    out = nc.declare_dram_parameter("out", [S, DK], f32, isOutput=True)

    NPRE, NCHEB, NITER = cfg["n_pre"], cfg["n_cheb"], cfg["num_iter"]

    lp = nc.allow_low_precision(reason="fp32r/bf16 datapath is intentional")
    lp.__enter__()
    with tile.TileContext(nc) as tc:
        consts = tc.alloc_tile_pool(name="consts", bufs=1)
        work = tc.alloc_tile_pool(name="work", bufs=1)
        vtp = tc.alloc_tile_pool(name="vtp", bufs=3)
        stage = tc.alloc_tile_pool(name="stage", bufs=2)
        setup_sb = tc.alloc_tile_pool(name="setup_sb", bufs=1)

        # =========== phase 0: constants + staging ========================
        setup_pp = tc.alloc_tile_pool(name="setup_pp", bufs=1, space="PSUM")

        trim_f = consts.tile([P, P], f32, name="trim_f")
        nc.sync.dma_start(out=trim_f[:], in_=trim[:])
        trim_r = consts.tile([P, P], f32r, name="trim_r")
        nc.vector.tensor_copy(trim_r[:], trim_f[:])
        trim_b = consts.tile([P, P], bf16, name="trim_b")
        nc.vector.tensor_copy(trim_b[:], trim_f[:])
        ident_f = consts.tile([P, P], f32, name="ident_f")
        nc.sync.dma_start(out=ident_f[:], in_=ident[:])
        ident_r = consts.tile([P, P], f32r, name="ident_r")
        nc.vector.tensor_copy(ident_r[:], ident_f[:])
        rinit_f = consts.tile([P, S], f32, name="rinit_f")
        nc.sync.dma_start(out=rinit_f[DK:P, :], in_=rinit[DK:P, :])

        cst_f = consts.tile([P, 4], f32, name="cst_f")
        nc.sync.dma_start(out=cst_f[:], in_=ccol[:])
        row_f = consts.tile([33, S], f32, name="row_f")
        nc.sync.dma_start(out=row_f[0:1, :], in_=crow[0:1, :])
        nc.sync.dma_start(out=row_f[32:33, :], in_=crow[32:33, :])
        zero_f = consts.tile([P, S], f32, name="zero_f")
        nc.vector.memset(zero_f[:], 0.0)
        ones64_r = consts.tile([DK, 2], f32r, name="ones64_r")
        nc.vector.tensor_copy(ones64_r[:], cst_f[0:DK, 2:4])
        onesrow_r = consts.tile([1, S], f32r, name="onesrow_r")
        nc.vector.tensor_copy(onesrow_r[:], row_f[0:1, :])
        selA_r = consts.tile([P, 1], f32r, name="selA_r")
        nc.vector.tensor_copy(selA_r[:], cst_f[:, 0:1])
        selB_r = consts.tile([P, 1], f32r, name="selB_r")
        nc.vector.tensor_copy(selB_r[:], cst_f[:, 1:2])
        selArow_r = consts.tile([1, P], f32r, name="selArow_r")
        nc.vector.tensor_copy(selArow_r[:], row_f[32:33, 0:P])
        selBrow_r = consts.tile([1, P], f32r, name="selBrow_r")
        nc.vector.tensor_tensor(selBrow_r[:], row_f[0:1, 0:P],
                                selArow_r[:], OP.subtract)
        negsp = consts.tile([P, 1], f32, name="negsp")
        nc.vector.memset(negsp[:], -SPARSITY)

        # weights -> fp32r tiles (staging f32 slot reused)
        wts = {}
        for nm, src in (("wq", wq), ("wk", wk), ("wv", wv)):
            for i in range(4):
                t = stage.tile([P, DK], f32, name=f"{nm}f{i}", tag="wstage")
                nc.sync.dma_start(out=t[:], in_=src[i * P:(i + 1) * P, :])
                tr = work.tile([P, DK], f32r, name=f"{nm}r{i}")
                nc.vector.tensor_copy(tr[:], t[:])
                wts[(nm, i)] = tr
        bias = {}
        for nm, src in (("bq", bq), ("bk", bk), ("bv", bv)):
            t = work.tile([DK, 1], f32, name=f"{nm}c")
            nc.sync.dma_start(out=t[:], in_=src[:].rearrange("(a b) -> a b", b=1))
            bias[nm] = t

        # x^T via PE transposes of 128x128 blocks, cast to fp32r
        xt_r = [setup_sb.tile([P, S], f32r, name=f"xtr{i}") for i in range(4)]
        for j in range(NT):
            xn = stage.tile([P, DM], f32, name=f"xn{j}", tag="xstage")
            nc.sync.dma_start(out=xn[:], in_=x[j * P:(j + 1) * P, :])
            for i in range(4):
                tp = setup_pp.tile([P, P], f32, name=f"xtp{j}_{i}", tag="tr",
                                   bufs=2)
                nc.tensor.transpose(tp[:], xn[:, i * P:(i + 1) * P], ident_f[:])
                nc.vector.tensor_copy(xt_r[i][:, j * P:(j + 1) * P], tp[:])

        # =========== phase 1: projections ================================
        def project(nm, bnm, sb_out):
            """sb_out [DK, S] f32  <-  W^T x^T + b (bias per-partition)."""
            for c in _chunks(0):
                ps = setup_pp.tile([DK, 512], f32, name=f"pj_{nm}_{c.start}",
                                   tag="pj")
                for i in range(4):
                    nc.tensor.matmul(ps[:], wts[(nm, i)][:], xt_r[i][:, c],
                                     start=(i == 0), stop=(i == 3))
                nc.scalar.activation(sb_out[:, c], ps[:], AF.Identity,
                                     bias=bias[bnm][:])

        kst_f = setup_sb.tile([DK, S], f32, name="kst_f")
        project("wk", "bk", kst_f)
        kst_r = work.tile([DK, S], f32r, name="kst_r")
        nc.vector.tensor_copy(kst_r[:], kst_f[:])
        qst_f = work.tile([DK, S], f32, name="qst_f")
        project("wq", "bq", qst_f)
        vt_f = setup_sb.tile([DK, S], f32, name="vt_f")
        project("wv", "bv", vt_f)

        # KsT duplicated on both partition halves (bf16) for row-packed C mm
        kst_b = work.tile([DK, S], bf16, name="kst_b")
        nc.vector.tensor_copy(kst_b[:], kst_f[:])
        kst_dup_b = work.tile([P, S], bf16, name="kst_dup_b")
        nc.sync.dma_start(out=kst_dup_b[0:DK, :], in_=kst_b[:])
        nc.sync.dma_start(out=kst_dup_b[DK:P, :], in_=kst_b[:])

        # natural-layout Ks / V via PE transpose
        ks_r, ks_b, v_r = [], [], []
        for i in range(NT):
            sl = slice(i * P, (i + 1) * P)
            tp = setup_pp.tile([P, DK], f32, name=f"kn{i}", tag="tr", bufs=2)
            nc.tensor.transpose(tp[:], kst_f[:, sl], ident_f[0:DK, 0:DK])
            kr = work.tile([P, DK], f32r, name=f"ksr{i}")
            nc.vector.tensor_copy(kr[:], tp[:])
            ks_r.append(kr)
            kb = work.tile([P, DK], bf16, name=f"ksb{i}")
            nc.vector.tensor_copy(kb[:], tp[:])
            ks_b.append(kb)
            tv = setup_pp.tile([P, DK], f32, name=f"vn{i}", tag="tr", bufs=2)
            nc.tensor.transpose(tv[:], vt_f[:, sl], ident_f[0:DK, 0:DK])
            vr = work.tile([P, DK], f32r, name=f"vr{i}")
            nc.vector.tensor_copy(vr[:], tv[:])
            v_r.append(vr)

        # v* = approx top eigvec of the final Gram via repeated squaring
        gps = setup_pp.tile([DK, DK], f32, name="gps", tag="gsq", bufs=2)
        for i in range(NT):
            nc.tensor.matmul(gps[:], ks_r[i][:], ks_r[i][:],
                             start=(i == 0), stop=(i == NT - 1))
        gcur = work.tile([DK, DK], f32r, name="g0")
        nc.vector.tensor_scalar_mul(gcur[:], gps[:], 0.0625)
        for q in range(5):
            g2ps = setup_pp.tile([DK, DK], f32, name=f"g2ps{q}", tag="gsq",
                                 bufs=2)
            nc.tensor.matmul(g2ps[:], gcur[:], gcur[:], start=True, stop=True)
            gnew = work.tile([DK, DK], f32r, name=f"g{q + 1}")
            nc.vector.tensor_copy(gnew[:], g2ps[:])
            gcur = gnew
        vst_ps = setup_pp.tile([DK, 2], f32, name="vst_ps", tag="tiny", bufs=2)
        nc.tensor.matmul(vst_ps[:], gcur[:], ones64_r[:], start=True, stop=True)
        vst = work.tile([DK, 2], f32r, name="vst")
        nc.vector.tensor_copy(vst[:], vst_ps[:])
        vsq = work.tile([DK, 2], f32r, name="vsq")
        nc.vector.tensor_tensor(vsq[:], vst[:], vst[:], OP.mult)
        nrm_ps = setup_pp.tile([1, 2], f32, name="nrm_ps", tag="tiny", bufs=2)
        nc.tensor.matmul(nrm_ps[:], vsq[:, 0:1], ones64_r[:], start=True, stop=True)
        nrm_sb = work.tile([1, 2], f32, name="nrm_sb")
        nc.scalar.sqrt(nrm_sb[:], nrm_ps[:])
        rnrm = work.tile([1, 2], f32r, name="rnrm")
        nc.vector.reciprocal(rnrm[:], nrm_sb[:])
        rnrm_bc_ps = setup_pp.tile([DK, 2], f32, name="rnrm_bc_ps", tag="tiny",
                                   bufs=2)
        nc.tensor.matmul(rnrm_bc_ps[:], selArow_r[:, 0:DK], rnrm[:],
                         start=True, stop=True)
        rnrm_bc = work.tile([DK, 2], f32r, name="rnrm_bc")
        nc.vector.tensor_copy(rnrm_bc[:], rnrm_bc_ps[:])
        vstn = work.tile([DK, 2], f32r, name="vstn")
        nc.vector.tensor_tensor(vstn[:], vst[:], rnrm_bc[:], OP.mult)
        vrow_ps = setup_pp.tile([1, DK], f32, name="vrow_ps", tag="tiny", bufs=2)
        nc.tensor.matmul(vrow_ps[:], vstn[:, 0:1], ident_r[0:DK, 0:DK],
                         start=True, stop=True)
        vrow = work.tile([1, DK], f32r, name="vrow")
        nc.vector.tensor_copy(vrow[:], vrow_ps[:])
        # VT0: rows 0-63 = v* (outer) ; rows 64-127 = random init
        v_cur = vtp.tile([P, S], bf16, name="vt0", tag="vt")
        for c in _chunks(0):
            vt0_ps = setup_pp.tile([DK, 512], f32, name=f"vt0_{c.start}",
                                   tag="pj")
            nc.tensor.matmul(vt0_ps[:], vrow[:], onesrow_r[:, c],
                             start=True, stop=True)
            nc.vector.tensor_copy(v_cur[0:DK, c], vt0_ps[:])
        nc.vector.tensor_copy(v_cur[DK:P, :], rinit_f[DK:P, :])
        setup_pp.release()
        setup_sb.release()
        stage.release()

        # =========== phase 2: power iteration for lam_max ================
        CONST_ETA = cfg.get("const_eta")
        spool = tc.alloc_tile_pool(name="spool", bufs=2)
        tpool = tc.alloc_tile_pool(name="tpool", bufs=2)
        upool = tc.alloc_tile_pool(name="upool", bufs=2)
        csp = tc.alloc_tile_pool(name="csp", bufs=2)
        if CONST_ETA is None:
            power_pp = tc.alloc_tile_pool(name="power_pp", bufs=1, space="PSUM")

            def apply_G(vcur, tag):
                """W psum tile [128, S]: rows 0-63 = G@V_A, 64-127 = G@V_B."""
                c_sbs = []
                for i in range(NT):
                    c0 = i * P
                    ca = power_pp.tile([P, S], f32, name=f"ca_{tag}_{i}", tag="ca")
                    cb = power_pp.tile([P, S], f32, name=f"cb_{tag}_{i}", tag="cb")
                    for c in _chunks(c0):
                        nc.tensor.matmul(ca[:, c], kst_dup_b[0:DK, c0:c0 + P],
                                         vcur[0:DK, c], start=True, stop=True)
                        nc.tensor.matmul(cb[:, c], kst_dup_b[DK:P, c0:c0 + P],
                                         vcur[DK:P, c], start=True, stop=True)
                    csa = csp.tile([P, S], bf16, name=f"csa_{tag}_{i}", tag="csa")
                    csb = csp.tile([P, S], bf16, name=f"csb_{tag}_{i}", tag="csb")
                    nc.vector.tensor_tensor(csa[:, c0:c0 + P], ca[:, c0:c0 + P],
                                            trim_b[:], OP.mult)
                    nc.vector.tensor_tensor(csb[:, c0:c0 + P], cb[:, c0:c0 + P],
                                            trim_b[:], OP.mult)
                    if c0 + P < S:
                        rest = slice(c0 + P, S)
                        if cfg["ccopy_eng"][i] == 'v':
                            nc.vector.tensor_copy(csa[:, rest], ca[:, rest])
                            nc.scalar.copy(csb[:, rest], cb[:, rest])
                        else:
                            nc.scalar.copy(csa[:, rest], ca[:, rest])
                            nc.vector.tensor_copy(csb[:, rest], cb[:, rest])
                    c_sbs.append((csa, csb))
                # consume each C tile immediately: i outer, chunk inner
                wps = power_pp.tile([P, S], f32, name=f"w_{tag}", tag="w")
                for i, (csa, csb) in enumerate(c_sbs):
                    for c in _chunks(i * P):
                        last_i = c.stop // P - 1
                        nc.tensor.matmul(wps[0:DK, c], ks_b[i][:], csa[:, c],
                                         start=(i == 0), stop=(i == last_i))
                        nc.tensor.matmul(wps[DK:P, c], ks_b[i][:], csb[:, c],
                                         start=(i == 0), stop=(i == last_i),
                                         tile_position=(0, 64))
                return wps

            def rq_rows(vcur, wps, tag, persist=True):
                """lamA/lamB rows [1,S] f32 (SBUF, partition 0) via colsum mms."""
                tmps = {}
                for which, pair in (("n", (vcur, wps)), ("d", (vcur, vcur))):
                    tmp = spool.tile([P, S], f32r, name=f"t{which}_{tag}",
                                     tag=f"t{which}", bufs=1)
                    nc.vector.tensor_tensor(tmp[:], pair[0][:], pair[1][:], OP.mult)
                    tmps[which] = tmp
                rows = {}
                for which in ("n", "d"):
                    for cand, sel in (("A", selA_r), ("B", selB_r)):
                        sb = spool.tile([1, S], f32, name=f"r{which}{cand}_{tag}",
                                        tag="rowtmp", bufs=5)
                        for c in _chunks(0):
                            rp = power_pp.tile([1, 512], f32,
                                               name=f"rp{which}{cand}_{tag}_{c.start}",
                                               tag="r")
                            nc.tensor.matmul(rp[:], sel[:], tmps[which][:, c],
                                             start=True, stop=True)
                            nc.scalar.copy(sb[:, c], rp[:])
                        rows[(which, cand)] = sb
                lams = []
                for cand in ("A", "B"):
                    rden = spool.tile([1, S], f32, name=f"rd{cand}_{tag}",
                                      tag="rowtmp", bufs=5)
                    nc.vector.reciprocal(rden[:], rows[("d", cand)][:])
                    if persist:
                        lam_t = work.tile([1, S], f32, name=f"lam{cand}_{tag}")
                    else:
                        lam_t = spool.tile([1, S], f32, name=f"lam{cand}_{tag}",
                                           tag="rowtmp2", bufs=3)
                    nc.vector.tensor_tensor(lam_t[:], rows[("n", cand)][:],
                                            rden[:], OP.mult)
                    lams.append(lam_t)
                return lams

            # pre-apps (plain power, scaled by 1/16)
            v_prev = None
            for it in range(NPRE):
                wps = apply_G(v_cur, f"pre{it}")
                if it == NPRE - 1:
                    lamA0, lamB0 = rq_rows(v_cur, wps, "pre")
                v_nxt = vtp.tile([P, S], bf16, name=f"vpre{it}", tag="vt")
                nc.vector.tensor_scalar_mul(v_nxt[:], wps[:], 0.0625)
                v_prev, v_cur = v_cur, v_nxt

            # rows: b = 0.95*lam0 ; bhalf = b/2 ; binv = 1/b
            bhalfA = spool.tile([1, S], f32r, name="bhalfA", tag="rowtmp2", bufs=3)
            nc.vector.tensor_scalar_mul(bhalfA[:], lamA0[:], 0.475)
            bhalfB = spool.tile([1, S], f32r, name="bhalfB", tag="rowtmp2", bufs=3)
            nc.vector.tensor_scalar_mul(bhalfB[:], lamB0[:], 0.475)
            binvA = spool.tile([1, S], f32r, name="binvA", tag="rowtmp2", bufs=3)
            bA = spool.tile([1, S], f32, name="bA", tag="rowtmp", bufs=5)
            nc.vector.tensor_scalar_mul(bA[:], lamA0[:], 0.95)
            nc.vector.reciprocal(binvA[:], bA[:])
            binvB = spool.tile([1, S], f32r, name="binvB", tag="rowtmp2", bufs=3)
            bB = spool.tile([1, S], f32, name="bB", tag="rowtmp", bufs=5)
            nc.vector.tensor_scalar_mul(bB[:], lamB0[:], 0.95)
            nc.vector.reciprocal(binvB[:], bB[:])

            def bcast_rows(rowA, rowB, nm):
                bsb = work.tile([P, S], bf16, name=f"bc_{nm}")
                for c in _chunks(0):
                    bps = power_pp.tile([P, 512], f32, name=f"bps_{nm}_{c.start}",
                                        tag="r")
                    nc.tensor.matmul(bps[:], selArow_r[:], rowA[:, c],
                                     start=True, stop=False)
                    nc.tensor.matmul(bps[:], selBrow_r[:], rowB[:, c],
                                     start=False, stop=True)
                    nc.vector.tensor_copy(bsb[:, c], bps[:])
                return bsb

            bhalf_bc = bcast_rows(bhalfA, bhalfB, "bhalf")
            binv_bc = bcast_rows(binvA, binvB, "binv")

            # Chebyshev apps; scaled recurrence U_{n+1}=(1/4)A U_n - (1/16)U_{n-1}
            for it in range(NCHEB):
                wps = apply_G(v_cur, f"ch{it}")
                bv = tpool.tile([P, S], bf16, name=f"bv{it}", tag="bv")
                nc.vector.tensor_tensor(bv[:], v_cur[:], bhalf_bc[:], OP.mult)
                t1 = tpool.tile([P, S], bf16, name=f"t1_{it}", tag="t1")
                nc.vector.tensor_tensor(t1[:], wps[:], bv[:], OP.subtract)
                t2 = tpool.tile([P, S], bf16, name=f"t2_{it}", tag="t2")
                nc.gpsimd.tensor_tensor(t2[:], t1[:], binv_bc[:], OP.mult)
                v_nxt = vtp.tile([P, S], bf16, name=f"vch{it}", tag="vt")
                if it == 0:
                    nc.vector.tensor_copy(v_nxt[:], t2[:])
                else:
                    nc.vector.scalar_tensor_tensor(v_nxt[:], v_prev[:], -0.0625,
                                                   t2[:], OP.mult, OP.add)
                v_prev, v_cur = v_cur, v_nxt

            # final Rayleigh quotient -> lambda -> eta
            wps = apply_G(v_cur, "fin")
            lamAf, lamBf = rq_rows(v_cur, wps, "fin", persist=False)
            lam_fin = spool.tile([1, S], f32, name="lam_fin", tag="rowtmp2", bufs=3)
            nc.vector.tensor_tensor(lam_fin[:], lamAf[:], lamBf[:], OP.max)
            nc.vector.tensor_tensor(lam_fin[:], lam_fin[:], lamA0[:], OP.max)
            nc.vector.tensor_tensor(lam_fin[:], lam_fin[:], lamB0[:], OP.max)

            eta_row = spool.tile([1, S], f32, name="eta_row", tag="rowtmp2", bufs=3)
            lam_eps = spool.tile([1, S], f32, name="lam_eps", tag="rowtmp", bufs=5)
            nc.vector.tensor_scalar_add(lam_eps[:], lam_fin[:], 1e-8)
            nc.vector.reciprocal(eta_row[:], lam_eps[:])
            negeta_row = spool.tile([1, S], f32r, name="negeta_row", tag="rowtmp2", bufs=3)
            nc.vector.tensor_scalar_mul(negeta_row[:], eta_row[:], -0.9)
            etap_row = spool.tile([1, S], f32r, name="etap_row", tag="rowtmp2", bufs=3)
            nc.vector.tensor_scalar_mul(etap_row[:], eta_row[:], 0.9)

            negeta_bc = work.tile([DK, S], f32, name="negeta_bc")
            for c in _chunks(0):
                nps = power_pp.tile([DK, 512], f32, name=f"nps_{c.start}", tag="r")
                nc.tensor.matmul(nps[:], selArow_r[:, 0:DK], negeta_row[:, c],
                                 start=True, stop=True)
                nc.vector.tensor_copy(negeta_bc[:, c], nps[:])

            eta_cols = []
            for j in range(NT):
                ep = power_pp.tile([P, 2], f32, name=f"etacp{j}", tag="r")
                nc.tensor.matmul(ep[:], etap_row[:, j * P:(j + 1) * P],
                                 onesrow_r[:, 0:2], start=True, stop=True)
                ec = work.tile([P, 1], f32, name=f"etac{j}")
                nc.vector.tensor_copy(ec[:], ep[:, 0:1])
                eta_cols.append(ec)
            power_pp.release()
        else:
            negeta_bc = work.tile([DK, S], f32, name="negeta_bc")
            nc.vector.memset(negeta_bc[:], -CONST_ETA)
            eta_cols = []
            for j in range(NT):
                ec = work.tile([P, 1], f32, name=f"etac{j}")
                nc.vector.memset(ec[:], CONST_ETA)
                eta_cols.append(ec)

        # =========== phase 3: beta-space ISTA ============================
        ista_pp = tc.alloc_tile_pool(name="ista_pp", bufs=1, space="PSUM")

        beta = []
        for i in range(NT):
            bt = work.tile([P, S], f32r, name=f"beta{i}")
            nc.vector.tensor_copy(bt[:], zero_f[:])
            beta.append(bt)

        for n in range(NITER):
            # KaT[k,t] = sum_s Ks[s,k] beta[s,t]  (256-col triangular chunks)
            kat = ista_pp.tile([DK, S], f32, name=f"kat{n}", tag="kat")
            for j in range(4):
                cs = slice(256 * j, 256 * (j + 1))
                smax = 2 * j + 2
                for i in range(smax):
                    nc.tensor.matmul(kat[:, cs], ks_r[i][:], beta[i][:, cs],
                                     start=(i == 0), stop=(i == smax - 1))
            # R' = QsT - eta*KaT  (fp32r; feeds the E matmul)
            tmp64 = spool.tile([DK, S], f32, name=f"tmp64_{n}", tag="tmp64", bufs=1)
            nc.vector.tensor_tensor(tmp64[:], kat[:], negeta_bc[:], OP.mult)
            rp_t = spool.tile([DK, S], f32r, name=f"rp{n}", tag="rp")
            nc.vector.tensor_tensor(rp_t[:], tmp64[:], qst_f[:], OP.add)

            for i in range(NT):
                c0 = i * P
                dp = ista_pp.tile([P, S], f32, name=f"d{n}_{i}", tag="dps",
                                  bufs=3)
                for c in _chunks(c0):
                    nc.tensor.matmul(dp[:, c], kst_r[:, c0:c0 + P], rp_t[:, c],
                                     start=True, stop=False)
                    nc.tensor.matmul(dp[:, c], ident_r[:], beta[i][:, c],
                                     start=False, stop=True)
                act = slice(c0, S)
                if cfg["thr_scheme"][i] == 'act':
                    u = upool.tile([P, S], f32, name=f"u{n}_{i}", tag="u")
                    nc.scalar.activation(u[:, act], dp[:, act], AF.Relu,
                                         bias=negsp[:])
                    w_ = upool.tile([P, S], f32, name=f"w{n}_{i}", tag="w")
                    nc.scalar.activation(w_[:, act], dp[:, act], AF.Relu,
                                         bias=negsp[:], scale=-1.0)
                    eng = nc.vector if cfg["combine_eng"][i] == 'v' else nc.gpsimd
                    eng.tensor_tensor(beta[i][:, act], u[:, act], w_[:, act],
                                      OP.subtract)
                else:
                    cl = upool.tile([P, S], f32, name=f"cl{n}_{i}", tag="u")
                    nc.vector.tensor_scalar(cl[:, act], dp[:, act], SPARSITY,
                                            -SPARSITY, OP.min, OP.max)
                    nc.vector.tensor_tensor(beta[i][:, act], dp[:, act],
                                            cl[:, act], OP.subtract)
                eng2 = nc.gpsimd if cfg["mask_eng"][i] == 'g' else nc.vector
                eng2.tensor_tensor(beta[i][:, c0:c0 + P], beta[i][:, c0:c0 + P],
                                   trim_r[:], OP.mult)

        # =========== phase 4: out[t,:] = eta_t * sum_s beta[s,t] V[s,:] ==
        for j in range(NT):
            ts_ = slice(j * P, (j + 1) * P)
            op = ista_pp.tile([P, DK], f32, name=f"ops{j}", tag="kat")
            for i in range(j + 1):
                nc.tensor.matmul(op[:], beta[i][:, ts_], v_r[i][:],
                                 start=(i == 0), stop=(i == j))
            osb = work.tile([P, DK], f32, name=f"osb{j}")
            nc.scalar.activation(osb[:], op[:], AF.Identity,
                                 scale=eta_cols[j][:])
            nc.sync.dma_start(out=out[ts_, :], in_=osb[:])

        ista_pp.release()
        for pool in (csp, upool, tpool, spool, vtp, work, consts):
            pool.release()
    lp.__exit__(None, None, None)

    return _spill_excess_waits(nc)


# ---------------------------------------------------------------------------
# PJRT SPMD runner (axon path), persistent across calls
# ---------------------------------------------------------------------------
def _make_runner(nc, n_cores=N_CORES):
    import jax
    from concourse import mybir
    from concourse.bass2jax import (_bass_exec_p, install_neuronx_cc_hook,
                                    partition_id_tensor)
    from jax.sharding import Mesh, PartitionSpec
    try:
        from jax.experimental.shard_map import shard_map
    except Exception:
        from jax.sharding import shard_map
    install_neuronx_cc_hook()
    partition_name = nc.partition_id_tensor.name if nc.partition_id_tensor else None
    in_names, out_names, out_avals, zero_outs = [], [], [], []
    for alloc in nc.m.functions[0].allocations:
        if not isinstance(alloc, mybir.MemoryLocationSet):
            continue
        name = alloc.memorylocations[0].name
        if alloc.kind == "ExternalInput":
            if name != partition_name:
                in_names.append(name)
        elif alloc.kind == "ExternalOutput":
            shp = tuple(alloc.tensor_shape)
            dt = mybir.dt.np(alloc.dtype)
            out_names.append(name)
            out_avals.append(jax.core.ShapedArray(shp, dt))
            zero_outs.append(np.zeros(shp, dt))
    n_params, n_outs = len(in_names), len(out_names)
    bind_in_names = list(in_names) + list(out_names)
    if partition_name is not None:
        bind_in_names.append(partition_name)

    def _body(*args):
        operands = list(args)
        if partition_name is not None:
            operands.append(partition_id_tensor())
        outs = _bass_exec_p.bind(
            *operands, out_avals=tuple(out_avals), in_names=tuple(bind_in_names),
            out_names=tuple(out_names), lowering_input_output_aliases=(),
            sim_require_finite=True, sim_require_nnan=True, nc=nc)
        return tuple(outs)

    devices = jax.devices()[:n_cores]
    mesh = Mesh(np.asarray(devices), ("core",))
    in_specs = (PartitionSpec("core"),) * (n_params + n_outs)
    out_specs = (PartitionSpec("core"),) * n_outs
    fn = jax.jit(shard_map(_body, mesh=mesh, in_specs=in_specs,
                           out_specs=out_specs, check_rep=False),
                 keep_unused=True)

    def call(in_maps):
        cin = [np.concatenate([np.asarray(m[name]) for m in in_maps], axis=0)
               for name in in_names]
        czero = [np.zeros((n_cores * z.shape[0], *z.shape[1:]), z.dtype)
                 for z in zero_outs]
        outs = [np.asarray(o) for o in fn(*cin, *czero)]
        per_core = []
        for c in range(n_cores):
            d = {}
            for i, name in enumerate(out_names):
                blk = outs[i].shape[0] // n_cores
                d[name] = outs[i][c * blk:(c + 1) * blk]
            per_core.append(d)
        return per_core

    call.fn = fn
    call.in_names = in_names
    return call


def _host_consts():
    rng = np.random.default_rng(1234)
    rinit = np.zeros((P, S), np.float32)
    rb = rng.standard_normal((DK, S)).astype(np.float32)
    rinit[DK:P] = rb / np.sqrt((rb * rb).sum(0, keepdims=True))
    sl, tl = np.arange(P)[:, None], np.arange(P)[None, :]
    trim = (tl >= sl).astype(np.float32)          # keep s<=t in [s,t] layout
    ident = np.eye(P, dtype=np.float32)
    ccol = np.zeros((P, 4), np.float32)
    ccol[0:DK, 0] = 1.0      # selA
    ccol[DK:P, 1] = 1.0      # selB
    ccol[:, 2] = 1.0         # ones
    ccol[:, 3] = 1.0     # ones (pair)
    crow = np.zeros((33, S), np.float32)
    crow[0, :] = 1.0         # ones row
    crow[32, 0:DK] = 1.0     # selA row
    return rinit, trim, ident, ccol, crow


def _make_in_maps(x, Wq, bq, Wk, bk, Wv, bv):
    rinit, trim, ident, ccol, crow = _host_consts()
    sc = np.float32(1.0 / np.sqrt(DK))
    shared = dict(
        wq=np.ascontiguousarray(Wq, np.float32) * sc,
        bq=np.ascontiguousarray(bq, np.float32) * sc,
        wk=np.ascontiguousarray(Wk, np.float32) * sc,
        bk=np.ascontiguousarray(bk, np.float32) * sc,
        wv=np.ascontiguousarray(Wv, np.float32),
        bv=np.ascontiguousarray(bv, np.float32),
        rinit=rinit, trim=trim, ident=ident, ccol=ccol, crow=crow,
    )
    return [dict(x=np.ascontiguousarray(x[c], np.float32), **shared)
            for c in range(N_CORES)]


def kernel(x, Wq, bq, Wk, bk, Wv, bv):
    global _RUNNER
    if _RUNNER is None:
        nc = _build_nc(CFG)
        _RUNNER = _make_runner(nc)
    in_maps = _make_in_maps(x, Wq, bq, Wk, bk, Wv, bv)
    res = _RUNNER(in_maps)
    return np.stack([res[c]["out"] for c in range(N_CORES)]).astype(np.float32)



# revision 28
# speedup vs baseline: 1.1567x; 1.1567x over previous
"""Trainium2 Bass kernel for nn_AttentionHead (sparse attention via per-timestep
ISTA with spectral step size).

Per batch element (data-parallel over 8 NeuronCores):
  Q/K/V projections; Qs = Q/sqrt(dk), Ks = K/sqrt(dk).
  lam_max(t) = top eigenvalue of the cumulative Gram G_t = sum_{s<=t} k_s k_s^T,
  computed with dual-candidate Chebyshev-accelerated power iteration (replaces
  eigvalsh; the output is insensitive to ~1% lambda error).
  eta_t = 0.9/(lam_max + 1e-8).
  ISTA runs in "beta space" (alpha = eta*beta), making the soft threshold the
  CONSTANT lambda=SPARSITY:
      beta <- mask * softthr_0.05( beta + Ks @ (QsT - eta*(Ks^T beta)) )
  out[t] = eta_t * (beta^T V)[t].

Matmuls run as float32r (full-rate reduced fp32, ~1.5e-4 rel err measured on
HW); the power phase is bf16. Causal structure is exploited: only the upper
triangle (s <= t) of every [S,S] object is computed.
"""
import numpy as np

B, S, DM, DK = 8, 1024, 512, 64
NUM_ITER, SPARSITY = 40, 0.05
N_CORES = 8
P = 128
NT = S // P   # 8 s-tiles

CFG = dict(
    const_eta=None,   # debug: float -> skip power phase, use constant eta
    n_pre=2,        # plain power apps before Chebyshev (RQ -> fixed b)
    n_cheb=6,       # Chebyshev-accelerated apps (single candidate)
    num_iter=NUM_ITER,
    # ISTA threshold scheme per s-tile (stock ops only; this walrus build
    # rejects custom-DVE instructions):
    #  'B': ident mm; 2 ACT relus; Pool combine
    #  'C': ident mm; DVE clamp(psum); DVE subtract(psum)
    #  'E': z=beta+dp on DVE; Pool clamp; Pool subtract
    #  'F': z=beta+dp on DVE; ACT relu; DVE min(sbuf); Pool combine
    thr_scheme=['C', 'B', 'C', 'B', 'B', 'C', 'B', 'B'],
    ccopy_eng=['v', 's', 'v', 's', 'v', 's', 'v', 's'],  # power C copy split
)

_RUNNER = None


# ---------------------------------------------------------------------------
# custom DVE op: out = softshrink(in0 + in1) with threshold s0 (=-s1).
# One Vector pass replaces {beta-add matmul, 2 relus, combine}.
# ---------------------------------------------------------------------------
def _register_dve_ops():
    import numpy as np
    from concourse import dve_ops
    from concourse.dve_spec import (Spec, Src0, Src1, C0, C1, minn, maxx,
                                    lower, _has_src1)
    from concourse.dve_uop import DveOpSpec

    by_name = {op.name: op for op in dve_ops.OPS}
    if "SOFTSHRINK_ADD_ANT" in by_name:
        return by_name["SOFTSHRINK_ADD_ANT"]

    def _ref(in0, in1, s0, s1, imm2):
        t = in0.astype(np.float32) + in1.astype(np.float32)
        return (t - np.clip(t, s1, s0)).astype(np.float32)

    z = Src0 + Src1
    spec = Spec(body=z - minn(maxx(z, C1), C0), reference=_ref)
    op = dve_ops.DveOp("SOFTSHRINK_ADD_ANT", spec, subdim=False, uops_sha={})
    dve_ops.OPS.append(op)
    dve_ops._SUB_OPCODE_FOR_NAME[op.name] = (
        dve_ops._CUSTOM_DVE_ROW_BASE + len(dve_ops.OPS) - 1)
    dve_ops.CUSTOM_DVE_SPECS[op.name] = spec
    for ver in ("v3", "v4"):
        try:
            s = DveOpSpec(name=op.name,
                          opcode=dve_ops.get_dve_sub_opcode(op.name),
                          uops=lower(spec, ver=ver), rd1_en=_has_src1(spec))
            op.uops_sha[ver] = s.sha(ver)
        except Exception:
            pass
    return op


def _chunks(c0, end=S, step=512):
    """512-aligned column chunks covering [c0, end)."""
    out = []
    c = c0
    while c < end:
        nxt = min(end, (c // step + 1) * step)
        out.append(slice(c, nxt))
        c = nxt
    return out


# ---------------------------------------------------------------------------
# wait-spill: this container's walrus allows ONE sem-wait per instruction.
# Move extras onto same-engine NoOps placed immediately before the offender.
# ---------------------------------------------------------------------------
def _spill_excess_waits(nc, max_waits=1):
    from concourse import mybir
    k = [0]
    for f in nc.m.functions:
        for bb in f.blocks:
            insts = list(bb.instructions)
            out_l, ch = [], False
            for ins_ in insts:
                si = ins_.sync_info
                w = list(si.on_wait) if si else []
                if len(w) > max_waits:
                    ch = True
                    si.on_wait = w[:max_waits]
                    for j in range(max_waits, len(w), max_waits):
                        k[0] += 1
                        nop = mybir.InstNoOp(name=f"ws-{k[0]}")
                        nop.engine = ins_.engine
                        nop.sync_info = type(si)(on_wait=w[j:j + max_waits],
                                                 on_update=[])
                        out_l.append(nop)
                out_l.append(ins_)
            if ch:
                bb.instructions = out_l
    return nc


# ---------------------------------------------------------------------------
# Bass program (one core = one batch element)
# ---------------------------------------------------------------------------
def _build_nc(cfg):
    import concourse.bass as bass
    import concourse.tile as tile
    from concourse import mybir

    f32 = mybir.dt.float32
    f32r = mybir.dt.float32r
    bf16 = mybir.dt.bfloat16
    AF = mybir.ActivationFunctionType
    OP = mybir.AluOpType

    nc = bass.Bass()

    x = nc.declare_dram_parameter("x", [S, DM], f32, isOutput=False)
    wq = nc.declare_dram_parameter("wq", [DM, DK], f32, isOutput=False)
    wk = nc.declare_dram_parameter("wk", [DM, DK], f32, isOutput=False)
    wv = nc.declare_dram_parameter("wv", [DM, DK], f32, isOutput=False)
    bq = nc.declare_dram_parameter("bq", [DK], f32, isOutput=False)
    bk = nc.declare_dram_parameter("bk", [DK], f32, isOutput=False)
    bv = nc.declare_dram_parameter("bv", [DK], f32, isOutput=False)
    trim = nc.declare_dram_parameter("trim", [P, P], f32, isOutput=False)
    ident = nc.declare_dram_parameter("ident", [P, P], f32, isOutput=False)
    ccol = nc.declare_dram_parameter("ccol", [P, 4], f32, isOutput=False)
    crow = nc.declare_dram_parameter("crow", [33, S], f32, isOutput=False)
    out = nc.declare_dram_parameter("out", [S, DK], f32, isOutput=True)

    NPRE, NCHEB, NITER = cfg["n_pre"], cfg["n_cheb"], cfg["num_iter"]

    lp = nc.allow_low_precision(reason="fp32r/bf16 datapath is intentional")
    lp.__enter__()
    with tile.TileContext(nc) as tc:
        consts = tc.alloc_tile_pool(name="consts", bufs=1)
        work = tc.alloc_tile_pool(name="work", bufs=1)
        vtp = tc.alloc_tile_pool(name="vtp", bufs=3)
        stage = tc.alloc_tile_pool(name="stage", bufs=2)
        setup_sb = tc.alloc_tile_pool(name="setup_sb", bufs=1)

        # =========== phase 0: constants + staging ========================
        setup_pp = tc.alloc_tile_pool(name="setup_pp", bufs=1, space="PSUM")

        trim_f = consts.tile([P, P], f32, name="trim_f")
        nc.sync.dma_start(out=trim_f[:], in_=trim[:])
        trim_r = consts.tile([P, P], f32r, name="trim_r")
        nc.vector.tensor_copy(trim_r[:], trim_f[:])
        trim_b = consts.tile([P, P], bf16, name="trim_b")
        nc.vector.tensor_copy(trim_b[:], trim_f[:])
        ident_f = consts.tile([P, P], f32, name="ident_f")
        nc.sync.dma_start(out=ident_f[:], in_=ident[:])
        ident_r = consts.tile([P, P], f32r, name="ident_r")
        nc.vector.tensor_copy(ident_r[:], ident_f[:])
        ident_b = consts.tile([P, P], bf16, name="ident_b")
        nc.vector.tensor_copy(ident_b[:], ident_f[:])

        cst_f = consts.tile([P, 4], f32, name="cst_f")
        nc.sync.dma_start(out=cst_f[:], in_=ccol[:])
        row_f = consts.tile([33, S], f32, name="row_f")
        nc.sync.dma_start(out=row_f[0:4, :], in_=crow[0:4, :])
        nc.sync.dma_start(out=row_f[32:33, :], in_=crow[32:33, :])
        zero_f = consts.tile([P, S], f32, name="zero_f")
        nc.vector.memset(zero_f[:], 0.0)
        ones64_r = consts.tile([DK, 2], f32r, name="ones64_r")
        nc.vector.tensor_copy(ones64_r[:], cst_f[0:DK, 2:4])
        onesrow_r = consts.tile([1, S], f32r, name="onesrow_r")
        nc.vector.tensor_copy(onesrow_r[:], row_f[0:1, :])
        selA_r = consts.tile([P, 1], f32r, name="selA_r")
        nc.vector.tensor_copy(selA_r[:], cst_f[:, 0:1])
        selArow_r = consts.tile([1, P], f32r, name="selArow_r")
        nc.vector.tensor_copy(selArow_r[:], row_f[32:33, 0:P])
        sel2_r = consts.tile([P, 2], f32r, name="sel2_r")
        nc.vector.tensor_copy(sel2_r[:], cst_f[:, 0:2])
        sel2T_f = consts.tile([2, P], f32, name="sel2T_f")
        nc.sync.dma_start(out=sel2T_f[:], in_=crow[2:4, 0:P])
        sel2T_r = consts.tile([2, P], f32r, name="sel2T_r")
        nc.vector.tensor_copy(sel2T_r[:], sel2T_f[:])
        negsp = consts.tile([P, 1], f32, name="negsp")
        nc.vector.memset(negsp[:], -SPARSITY)

        # weights -> fp32r tiles (staging f32 slot reused)
        wts = {}
        for nm, src in (("wq", wq), ("wk", wk), ("wv", wv)):
            for i in range(4):
                t = stage.tile([P, DK], f32, name=f"{nm}f{i}", tag="wstage")
                nc.sync.dma_start(out=t[:], in_=src[i * P:(i + 1) * P, :])
                tr = work.tile([P, DK], f32r, name=f"{nm}r{i}")
                nc.vector.tensor_copy(tr[:], t[:])
                wts[(nm, i)] = tr
        bias = {}
        for nm, src in (("bq", bq), ("bk", bk), ("bv", bv)):
            t = work.tile([DK, 1], f32, name=f"{nm}c")
            nc.sync.dma_start(out=t[:], in_=src[:].rearrange("(a b) -> a b", b=1))
            bias[nm] = t

        # x^T via PE transposes of 128x128 blocks, cast to fp32r
        xt_r = [setup_sb.tile([P, S], f32r, name=f"xtr{i}") for i in range(4)]
        for j in range(NT):
            xn = stage.tile([P, DM], f32, name=f"xn{j}", tag="xstage")
            nc.sync.dma_start(out=xn[:], in_=x[j * P:(j + 1) * P, :])
            for i in range(4):
                tp = setup_pp.tile([P, P], f32, name=f"xtp{j}_{i}", tag="tr",
                                   bufs=2)
                nc.tensor.transpose(tp[:], xn[:, i * P:(i + 1) * P], ident_f[:])
                nc.vector.tensor_copy(xt_r[i][:, j * P:(j + 1) * P], tp[:])

        # =========== phase 1: projections ================================
        def project(nm, bnm, sb_out, dup=False):
            """sb_out [DK, S] f32  <-  W^T x^T + b (bias per-partition).
            dup=True: sb_out is [P, 512]; chunk 0 -> rows 0:64, chunk 1 ->
            rows 64:128 (the dup layout used by the ISTA rp pipeline)."""
            for c in _chunks(0):
                ps = setup_pp.tile([DK, 512], f32, name=f"pj_{nm}_{c.start}",
                                   tag="pj")
                for i in range(4):
                    nc.tensor.matmul(ps[:], wts[(nm, i)][:], xt_r[i][:, c],
                                     start=(i == 0), stop=(i == 3))
                if dup:
                    rows = slice(0, DK) if c.start == 0 else slice(DK, P)
                    nc.scalar.activation(sb_out[rows, :], ps[:], AF.Identity,
                                         bias=bias[bnm][:])
                else:
                    nc.scalar.activation(sb_out[:, c], ps[:], AF.Identity,
                                         bias=bias[bnm][:])

        kst_f = setup_sb.tile([DK, S], f32, name="kst_f")
        project("wk", "bk", kst_f)
        # KsT duplicated on both partition halves (f32r) for the dp matmuls
        kst_dup_r = work.tile([P, S], f32r, name="kst_dup_r")
        nc.vector.tensor_copy(kst_dup_r[0:DK, :], kst_f[:])
        nc.scalar.copy(kst_dup_r[DK:P, :], kst_f[:])
        # QsT in dup layout [128, 512] f32r
        qst_dup = work.tile([P, 512], f32r, name="qst_dup")
        project("wq", "bq", qst_dup, dup=True)
        vt_f = setup_sb.tile([DK, S], f32, name="vt_f")
        project("wv", "bv", vt_f)

        # KsT duplicated on both partition halves (bf16) for row-packed C mm
        kst_b = work.tile([DK, S], bf16, name="kst_b")
        nc.vector.tensor_copy(kst_b[:], kst_f[:])
        kst_dup_b = work.tile([P, S], bf16, name="kst_dup_b")
        nc.sync.dma_start(out=kst_dup_b[0:DK, :], in_=kst_b[:])
        nc.sync.dma_start(out=kst_dup_b[DK:P, :], in_=kst_b[:])

        # natural-layout Ks / V via PE transpose
        ks_r, ks_b, v_r = [], [], []
        for i in range(NT):
            sl = slice(i * P, (i + 1) * P)
            tp = setup_pp.tile([P, DK], f32, name=f"kn{i}", tag="tr", bufs=2)
            nc.tensor.transpose(tp[:], kst_f[:, sl], ident_f[0:DK, 0:DK])
            kr = work.tile([P, DK], f32r, name=f"ksr{i}")
            nc.vector.tensor_copy(kr[:], tp[:])
            ks_r.append(kr)
            kb = work.tile([P, DK], bf16, name=f"ksb{i}")
            nc.vector.tensor_copy(kb[:], tp[:])
            ks_b.append(kb)
            tv = setup_pp.tile([P, DK], f32, name=f"vn{i}", tag="tr", bufs=2)
            nc.tensor.transpose(tv[:], vt_f[:, sl], ident_f[0:DK, 0:DK])
            vr = work.tile([P, DK], bf16, name=f"vb{i}")
            nc.vector.tensor_copy(vr[:], tv[:])
            v_r.append(vr)

        # v* = approx top eigvec of the final Gram via repeated squaring
        gps = setup_pp.tile([DK, DK], f32, name="gps", tag="gsq", bufs=2)
        for i in range(NT):
            nc.tensor.matmul(gps[:], ks_r[i][:], ks_r[i][:],
                             start=(i == 0), stop=(i == NT - 1))
        gcur = work.tile([DK, DK], f32r, name="g0")
        nc.vector.tensor_scalar_mul(gcur[:], gps[:], 0.0625)
        for q in range(5):
            g2ps = setup_pp.tile([DK, DK], f32, name=f"g2ps{q}", tag="gsq",
                                 bufs=2)
            nc.tensor.matmul(g2ps[:], gcur[:], gcur[:], start=True, stop=True)
            gnew = work.tile([DK, DK], f32r, name=f"g{q + 1}")
            nc.vector.tensor_copy(gnew[:], g2ps[:])
            gcur = gnew
        vst_ps = setup_pp.tile([DK, 2], f32, name="vst_ps", tag="tiny", bufs=2)
        nc.tensor.matmul(vst_ps[:], gcur[:], ones64_r[:], start=True, stop=True)
        vst = work.tile([DK, 2], f32r, name="vst")
        nc.vector.tensor_copy(vst[:], vst_ps[:])
        vsq = work.tile([DK, 2], f32r, name="vsq")
        nc.vector.tensor_tensor(vsq[:], vst[:], vst[:], OP.mult)
        nrm_ps = setup_pp.tile([1, 2], f32, name="nrm_ps", tag="tiny", bufs=2)
        nc.tensor.matmul(nrm_ps[:], vsq[:, 0:1], ones64_r[:], start=True, stop=True)
        nrm_sb = work.tile([1, 2], f32, name="nrm_sb")
        nc.scalar.sqrt(nrm_sb[:], nrm_ps[:])
        rnrm = work.tile([1, 2], f32r, name="rnrm")
        nc.vector.reciprocal(rnrm[:], nrm_sb[:])
        rnrm_bc_ps = setup_pp.tile([DK, 2], f32, name="rnrm_bc_ps", tag="tiny",
                                   bufs=2)
        nc.tensor.matmul(rnrm_bc_ps[:], selArow_r[:, 0:DK], rnrm[:],
                         start=True, stop=True)
        rnrm_bc = work.tile([DK, 2], f32r, name="rnrm_bc")
        nc.vector.tensor_copy(rnrm_bc[:], rnrm_bc_ps[:])
        vstn = work.tile([DK, 2], f32r, name="vstn")
        nc.vector.tensor_tensor(vstn[:], vst[:], rnrm_bc[:], OP.mult)
        vrow_ps = setup_pp.tile([1, DK], f32, name="vrow_ps", tag="tiny", bufs=2)
        nc.tensor.matmul(vrow_ps[:], vstn[:, 0:1], ident_r[0:DK, 0:DK],
                         start=True, stop=True)
        vrow = work.tile([1, DK], f32r, name="vrow")
        nc.vector.tensor_copy(vrow[:], vrow_ps[:])
        # VT0 (single candidate): both dup halves = v* broadcast over t
        v_cur = vtp.tile([P, 512], bf16, name="vt0", tag="vt")
        vt0_ps = setup_pp.tile([DK, 512], f32, name="vt0_ps", tag="pj")
        nc.tensor.matmul(vt0_ps[:], vrow[:], onesrow_r[:, 0:512],
                         start=True, stop=True)
        nc.vector.tensor_copy(v_cur[0:DK, :], vt0_ps[:])
        nc.scalar.copy(v_cur[DK:P, :], vt0_ps[:])
        setup_pp.release()
        setup_sb.release()
        stage.release()

        # =========== phase 2: power iteration for lam_max ================
        CONST_ETA = cfg.get("const_eta")
        spool = tc.alloc_tile_pool(name="spool", bufs=2)
        tpool = tc.alloc_tile_pool(name="tpool", bufs=2)
        upool = tc.alloc_tile_pool(name="upool", bufs=2)
        csp = tc.alloc_tile_pool(name="csp", bufs=2)
        if CONST_ETA is None:
            power_pp = tc.alloc_tile_pool(name="power_pp", bufs=1, space="PSUM")

            def apply_G(vcur, tag):
                """W dup psum [128, 512] <- per-column truncated-Gram apply of
                vcur (dup layout, single candidate)."""
                c_sbs = []
                for i in range(NT):
                    c0 = i * P
                    ca = power_pp.tile([P, S], f32, name=f"ca_{tag}_{i}",
                                       tag="ca", bufs=2)
                    for c in _chunks(c0):
                        rows = (slice(0, DK) if c.start < 512
                                else slice(DK, P))
                        off = 0 if c.start < 512 else 512
                        nc.tensor.matmul(ca[:, c], kst_dup_b[rows, c0:c0 + P],
                                         vcur[rows, c.start - off:c.stop - off],
                                         start=True, stop=True)
                    csa = csp.tile([P, S], bf16, name=f"csa_{tag}_{i}",
                                   tag="csa", bufs=2)
                    nc.vector.tensor_tensor(csa[:, c0:c0 + P], ca[:, c0:c0 + P],
                                            trim_b[:], OP.mult)
                    if c0 + P < S:
                        rest = slice(c0 + P, S)
                        if cfg["ccopy_eng"][i] == 'v':
                            nc.vector.tensor_copy(csa[:, rest], ca[:, rest])
                        else:
                            nc.scalar.copy(csa[:, rest], ca[:, rest])
                    c_sbs.append(csa)
                # consume each C tile immediately: i outer, chunk inner.
                # wps dup [128, 512]: rows 0:64 <- cols 0:512 (group zeroed by
                # the full-width i=0 matmul), rows 64:128 <- cols 512:1024.
                wps = power_pp.tile([P, 512], f32, name=f"w_{tag}", tag="w",
                                    bufs=2)
                for i, csa in enumerate(c_sbs):
                    for c in _chunks(i * P):
                        last_i = c.stop // P - 1
                        if c.start < 512:
                            o, tp = wps[0:DK, c], None
                        else:
                            o = wps[DK:P, c.start - 512:c.stop - 512]
                            tp = (0, DK)
                        nc.tensor.matmul(o, ks_b[i][:], csa[:, c],
                                         start=(i == 0), stop=(i == last_i),
                                         tile_position=tp,
                                         skip_group_check=True)
                return wps

            def rq_rows(vcur, wps, tag):
                """[2, 512] f32 Rayleigh-quotient rows: n/d per column-half."""
                tmpn = spool.tile([P, 512], f32r, name=f"tn_{tag}", tag="tn",
                                  bufs=1)
                nc.vector.tensor_tensor(tmpn[:], vcur[:], wps[:], OP.mult)
                tmpd = spool.tile([P, 512], f32r, name=f"td_{tag}", tag="td",
                                  bufs=1)
                nc.gpsimd.tensor_tensor(tmpd[:], vcur[:], vcur[:], OP.mult)
                nps = power_pp.tile([2, 512], f32, name=f"nps_{tag}", tag="rq",
                                    bufs=2)
                nc.tensor.matmul(nps[:], sel2_r[:], tmpn[:], start=True,
                                 stop=True)
                dps = power_pp.tile([2, 512], f32, name=f"dps_{tag}", tag="rq",
                                    bufs=2)
                nc.tensor.matmul(dps[:], sel2_r[:], tmpd[:], start=True,
                                 stop=True)
                rden = spool.tile([2, 512], f32r, name=f"rden_{tag}",
                                  tag="row", bufs=6)
                nc.vector.reciprocal(rden[:], dps[:])
                rq = spool.tile([2, 512], f32, name=f"rq_{tag}", tag="row",
                                bufs=6)
                nc.vector.tensor_tensor(rq[:], nps[:], rden[:], OP.mult)
                return rq

            # pre-apps (plain power, scaled by 1/16)
            v_prev = None
            for it in range(NPRE):
                wps = apply_G(v_cur, f"pre{it}")
                if it == NPRE - 1:
                    lam0 = work.tile([2, 512], f32, name="lam0")
                    nc.vector.tensor_copy(lam0[:], rq_rows(v_cur, wps, "pre")[:])
                v_nxt = vtp.tile([P, 512], bf16, name=f"vpre{it}", tag="vt")
                nc.scalar.activation(v_nxt[:], wps[:], AF.Identity,
                                     scale=0.0625)
                v_prev, v_cur = v_cur, v_nxt

            # b = 0.95*lam0 ; binv broadcast to dup layout (bf16)
            b_row = work.tile([2, 512], f32r, name="b_row")
            nc.vector.tensor_scalar_mul(b_row[:], lam0[:], 0.95)
            binv_row = spool.tile([2, 512], f32r, name="binv_row", tag="row",
                                  bufs=6)
            nc.vector.reciprocal(binv_row[:], b_row[:])
            bps = power_pp.tile([P, 512], f32, name="bps", tag="rq", bufs=2)
            nc.tensor.matmul(bps[:], sel2T_r[:], binv_row[:], start=True,
                             stop=True)
            binv_dup = work.tile([P, 512], bf16, name="binv_dup")
            nc.vector.tensor_copy(binv_dup[:], bps[:])

            # Chebyshev apps on the binv-scaled operator:
            #   W~ = G(binv*v) = binv*(G v);  v' = W~ - 0.5 v - (1/16) v_prev
            def scaled_apply(vc, tag):
                vs = tpool.tile([P, 512], bf16, name=f"vs_{tag}", tag="vs")
                nc.vector.tensor_tensor(vs[:], vc[:], binv_dup[:], OP.mult)
                return apply_G(vs, tag)

            for it in range(NCHEB):
                wps = scaled_apply(v_cur, f"ch{it}")
                t1 = tpool.tile([P, 512], bf16, name=f"t1_{it}", tag="t1")
                nc.vector.scalar_tensor_tensor(t1[:], v_cur[:], -0.5, wps[:],
                                               OP.mult, OP.add)
                v_nxt = vtp.tile([P, 512], bf16, name=f"vch{it}", tag="vt")
                if it == 0:
                    nc.scalar.copy(v_nxt[:], t1[:])
                else:
                    vp16 = tpool.tile([P, 512], bf16, name=f"vp16_{it}",
                                      tag="vp16")
                    nc.gpsimd.tensor_scalar(vp16[:], v_prev[:], -0.0625,
                                            None, OP.mult)
                    nc.gpsimd.tensor_tensor(v_nxt[:], vp16[:], t1[:], OP.add)
                v_prev, v_cur = v_cur, v_nxt

            # final Rayleigh quotient (scaled apply -> unscale by b) -> eta
            wps = scaled_apply(v_cur, "fin")
            rqf = rq_rows(v_cur, wps, "fin")
            lam_fin = spool.tile([2, 512], f32, name="lam_fin", tag="row",
                                 bufs=6)
            nc.vector.tensor_tensor(lam_fin[:], rqf[:], b_row[:], OP.mult)
            nc.vector.tensor_tensor(lam_fin[:], lam_fin[:], lam0[:], OP.max)

            lam_eps = spool.tile([2, 512], f32, name="lam_eps", tag="row",
                                 bufs=6)
            nc.vector.tensor_scalar_add(lam_eps[:], lam_fin[:], 1e-8)
            eta_row = spool.tile([2, 512], f32, name="eta_row", tag="row",
                                 bufs=6)
            nc.vector.reciprocal(eta_row[:], lam_eps[:])
            negeta_row = spool.tile([2, 512], f32r, name="negeta_row",
                                    tag="row2", bufs=2)
            nc.vector.tensor_scalar_mul(negeta_row[:], eta_row[:], -0.9)
            etap_row = spool.tile([2, 512], f32r, name="etap_row", tag="row2",
                                  bufs=2)
            nc.vector.tensor_scalar_mul(etap_row[:], eta_row[:], 0.9)

            negeta_dup = work.tile([P, 512], f32, name="negeta_dup")
            nps2 = power_pp.tile([P, 512], f32, name="nps2", tag="rq", bufs=2)
            nc.tensor.matmul(nps2[:], sel2T_r[:], negeta_row[:], start=True,
                             stop=True)
            nc.vector.tensor_copy(negeta_dup[:], nps2[:])

            eta_cols = [None] * NT
            for q in range(4):
                tc_ = slice(q * P, q * P + P)
                ep = power_pp.tile([P, 2], f32r, name=f"etacp{q}", tag="rq",
                                   bufs=2)
                nc.tensor.transpose(ep[:], etap_row[:, tc_], ident_r[0:2, 0:2])
                for h in range(2):
                    ec = work.tile([P, 1], f32, name=f"etac{q}_{h}")
                    nc.vector.tensor_copy(ec[:], ep[:, h:h + 1])
                    eta_cols[h * 4 + q] = ec
            power_pp.release()
        else:
            negeta_dup = work.tile([P, 512], f32, name="negeta_dup")
            nc.vector.memset(negeta_dup[:], -CONST_ETA)
            eta_cols = []
            for j in range(NT):
                ec = work.tile([P, 1], f32, name=f"etac{j}")
                nc.vector.memset(ec[:], CONST_ETA)
                eta_cols.append(ec)

        # =========== phase 3: beta-space ISTA ============================
        # dup layout: [128, 512] tiles, rows 0:64 <-> cols 0:512, rows
        # 64:128 <-> cols 512:1024 (full-lane DVE/Pool ops on [DK, S] data).
        ista_pp = tc.alloc_tile_pool(name="ista_pp", bufs=1, space="PSUM")

        SCHEME = cfg["thr_scheme"]
        beta_cur = []
        for i in range(NT):
            bt = work.tile([P, S], bf16, name=f"beta{i}")
            nc.vector.tensor_copy(bt[:], zero_f[:])
            beta_cur.append(bt)

        def dp_chunks(i):
            """(orig cols, k-rows, rp dup cols); sub-256 chunks padded."""
            c0 = i * P
            if c0 < 512:
                a0 = min(c0, 256)
                return [(slice(a0, 512), slice(0, DK), slice(a0, 512)),
                        (slice(512, S), slice(DK, P), slice(0, 512))]
            a0 = min(c0 - 512, 256)
            return [(slice(a0 + 512, S), slice(DK, P), slice(a0, 512))]

        TILE_ORDER = cfg.get("tile_order", (4, 0, 5, 1, 6, 2, 7, 3))
        for n in range(NITER):
            # KaT[k,t] = sum_s Ks[s,k] beta[s,t].  psum [128, 1024] with the
            # four 256-col chunks at (rows 0:64 | 64:128) x (cols 0:256 |
            # 512:768): each accumulation group has its own 2KB psum zero
            # region, so the groups may interleave freely on PE.  Matmuls are
            # emitted in tile-readiness order so PE never queues ready work
            # behind a late beta; start/stop go on the first/last emitted
            # matmul of each chunk.
            kat = ista_pp.tile([P, S], f32, name=f"kat{n}", tag="kat",
                               bufs=1)
            kat_sched = []
            for i in TILE_ORDER:
                for j in range(4):
                    if i <= 2 * j + 1:
                        kat_sched.append((i, j))
            first_of = {}
            last_of = {}
            for (i, j) in kat_sched:
                first_of.setdefault(j, (i, j))
                last_of[j] = (i, j)
            for (i, j) in kat_sched:
                cs = slice(256 * j, 256 * (j + 1))
                oc0 = 512 * (j % 2)
                orows = slice(0, DK) if j < 2 else slice(DK, P)
                tp = None if j < 2 else (0, DK)
                nc.tensor.matmul(kat[orows, oc0:oc0 + 256], ks_b[i][:],
                                 beta_cur[i][:, cs],
                                 start=(first_of[j] == (i, j)),
                                 stop=(last_of[j] == (i, j)),
                                 tile_position=tp, skip_group_check=True)
            # rp = QsT - eta*KaT  (dup layout, full-lane); kat read via the
            # strided blocks-0,2 view to skip its per-bank padding
            kat3 = kat[:].rearrange("p (b c) -> p b c", c=256)[:, 0:4:2, :]
            tmp = spool.tile([P, 512], f32r, name=f"tmp{n}", tag="tmp", bufs=2)
            nc.vector.tensor_tensor(
                tmp[:].rearrange("p (b c) -> p b c", c=256), kat3,
                negeta_dup[:].rearrange("p (b c) -> p b c", c=256), OP.mult)
            rp = spool.tile([P, 512], f32r, name=f"rp{n}", tag="rp", bufs=2)
            nc.gpsimd.tensor_tensor(rp[:], tmp[:], qst_dup[:], OP.add)

            for i in TILE_ORDER:
                c0 = i * P
                act = slice(c0, S)
                sch = SCHEME[i]
                with_ident = sch in ('B', 'C')
                dp = ista_pp.tile([P, S], f32, name=f"d{n}_{i}", tag="dps",
                                  bufs=3)
                for (ocs, rows, dcs) in dp_chunks(i):
                    nc.tensor.matmul(dp[:, ocs], kst_dup_r[rows, c0:c0 + P],
                                     rp[rows, dcs],
                                     start=True, stop=not with_ident)
                    if with_ident:
                        nc.tensor.matmul(dp[:, ocs], ident_b[:],
                                         beta_cur[i][:, ocs],
                                         start=False, stop=True)
                bt = beta_cur[i]
                if sch == 'B':
                    u = upool.tile([P, S], f32, name=f"u{n}_{i}", tag="u",
                                   bufs=3)
                    nc.scalar.activation(u[:, act], dp[:, act], AF.Relu,
                                         bias=negsp[:])
                    w_ = upool.tile([P, S], f32, name=f"w{n}_{i}", tag="w",
                                    bufs=3)
                    nc.scalar.activation(w_[:, act], dp[:, act], AF.Relu,
                                         bias=negsp[:], scale=-1.0)
                    nc.gpsimd.tensor_tensor(bt[:, act], u[:, act],
                                            w_[:, act], OP.subtract)
                elif sch == 'C':
                    cl = upool.tile([P, S], f32r, name=f"cl{n}_{i}", tag="u",
                                    bufs=3)
                    nc.vector.tensor_scalar(cl[:, act], dp[:, act], SPARSITY,
                                            -SPARSITY, OP.min, OP.max)
                    nc.vector.tensor_tensor(bt[:, act], dp[:, act],
                                            cl[:, act], OP.subtract)
                elif sch == 'E':
                    z = upool.tile([P, S], f32r, name=f"z{n}_{i}", tag="z",
                                   bufs=3)
                    nc.vector.tensor_tensor(z[:, act], dp[:, act],
                                            bt[:, act], OP.add)
                    cl = upool.tile([P, S], f32r, name=f"cl{n}_{i}", tag="u",
                                    bufs=3)
                    nc.gpsimd.tensor_scalar(cl[:, act], z[:, act], SPARSITY,
                                            -SPARSITY, OP.min, OP.max)
                    nc.gpsimd.tensor_tensor(bt[:, act], z[:, act],
                                            cl[:, act], OP.subtract)
                else:   # 'F'
                    z = upool.tile([P, S], f32r, name=f"z{n}_{i}", tag="z",
                                   bufs=3)
                    nc.vector.tensor_tensor(z[:, act], dp[:, act],
                                            bt[:, act], OP.add)
                    u = upool.tile([P, S], f32, name=f"u{n}_{i}", tag="u",
                                   bufs=3)
                    nc.scalar.activation(u[:, act], z[:, act], AF.Relu,
                                         bias=negsp[:])
                    w_ = upool.tile([P, S], f32r, name=f"w{n}_{i}", tag="w",
                                    bufs=3)
                    nc.vector.tensor_scalar(w_[:, act], z[:, act], SPARSITY,
                                            0.0, OP.add, OP.min)
                    nc.gpsimd.tensor_tensor(bt[:, act], u[:, act],
                                            w_[:, act], OP.add)
                nc.gpsimd.tensor_tensor(bt[:, c0:c0 + P], bt[:, c0:c0 + P],
                                        trim_b[:], OP.mult)

        # =========== phase 4: out[t,:] = eta_t * sum_s beta[s,t] V[s,:] ==
        for j in range(NT):
            ts_ = slice(j * P, (j + 1) * P)
            op = ista_pp.tile([P, DK], f32, name=f"ops{j}", tag="dps", bufs=3)
            for i in range(j + 1):
                nc.tensor.matmul(op[:], beta_cur[i][:, ts_], v_r[i][:],
                                 start=(i == 0), stop=(i == j))
            osb = work.tile([P, DK], f32, name=f"osb{j}")
            nc.scalar.activation(osb[:], op[:], AF.Identity,
                                 scale=eta_cols[j][:])
            nc.sync.dma_start(out=out[ts_, :], in_=osb[:])

        ista_pp.release()
        for pool in (csp, upool, tpool, spool, vtp, work, consts):
            pool.release()
    lp.__exit__(None, None, None)

    return _spill_excess_waits(nc)


# ---------------------------------------------------------------------------
# PJRT SPMD runner (axon path), persistent across calls
# ---------------------------------------------------------------------------
def _make_runner(nc, n_cores=N_CORES):
    import jax
    from concourse import mybir
    from concourse.bass2jax import (_bass_exec_p, install_neuronx_cc_hook,
                                    partition_id_tensor)
    from jax.sharding import Mesh, PartitionSpec
    try:
        from jax.experimental.shard_map import shard_map
    except Exception:
        from jax.sharding import shard_map
    install_neuronx_cc_hook()
    partition_name = nc.partition_id_tensor.name if nc.partition_id_tensor else None
    in_names, out_names, out_avals, zero_outs = [], [], [], []
    for alloc in nc.m.functions[0].allocations:
        if not isinstance(alloc, mybir.MemoryLocationSet):
            continue
        name = alloc.memorylocations[0].name
        if alloc.kind == "ExternalInput":
            if name != partition_name:
                in_names.append(name)
        elif alloc.kind == "ExternalOutput":
            shp = tuple(alloc.tensor_shape)
            dt = mybir.dt.np(alloc.dtype)
            out_names.append(name)
            out_avals.append(jax.core.ShapedArray(shp, dt))
            zero_outs.append(np.zeros(shp, dt))
    n_params, n_outs = len(in_names), len(out_names)
    bind_in_names = list(in_names) + list(out_names)
    if partition_name is not None:
        bind_in_names.append(partition_name)

    def _body(*args):
        operands = list(args)
        if partition_name is not None:
            operands.append(partition_id_tensor())
        outs = _bass_exec_p.bind(
            *operands, out_avals=tuple(out_avals), in_names=tuple(bind_in_names),
            out_names=tuple(out_names), lowering_input_output_aliases=(),
            sim_require_finite=True, sim_require_nnan=True, nc=nc)
        return tuple(outs)

    devices = jax.devices()[:n_cores]
    mesh = Mesh(np.asarray(devices), ("core",))
    in_specs = (PartitionSpec("core"),) * (n_params + n_outs)
    out_specs = (PartitionSpec("core"),) * n_outs
    fn = jax.jit(shard_map(_body, mesh=mesh, in_specs=in_specs,
                           out_specs=out_specs, check_rep=False),
                 keep_unused=True)

    def call(in_maps):
        cin = [np.concatenate([np.asarray(m[name]) for m in in_maps], axis=0)
               for name in in_names]
        czero = [np.zeros((n_cores * z.shape[0], *z.shape[1:]), z.dtype)
                 for z in zero_outs]
        outs = [np.asarray(o) for o in fn(*cin, *czero)]
        per_core = []
        for c in range(n_cores):
            d = {}
            for i, name in enumerate(out_names):
                blk = outs[i].shape[0] // n_cores
                d[name] = outs[i][c * blk:(c + 1) * blk]
            per_core.append(d)
        return per_core

    call.fn = fn
    call.in_names = in_names
    return call


def _host_consts():
    sl, tl = np.arange(P)[:, None], np.arange(P)[None, :]
    trim = (tl >= sl).astype(np.float32)          # keep s<=t in [s,t] layout
    ident = np.eye(P, dtype=np.float32)
    ccol = np.zeros((P, 4), np.float32)
    ccol[0:DK, 0] = 1.0      # selA
    ccol[DK:P, 1] = 1.0      # selB
    ccol[:, 2] = 1.0         # ones
    ccol[:, 3] = 1.0     # ones (pair)
    crow = np.zeros((33, S), np.float32)
    crow[0, :] = 1.0         # ones row (partition 0)
    crow[1, :] = 1.0         # ones row (partition 1)
    crow[2, 0:DK] = 1.0      # sel2T row 0: partitions 0:64
    crow[3, DK:P] = 1.0      # sel2T row 1: partitions 64:128
    crow[32, 0:DK] = 1.0     # selA row
    return trim, ident, ccol, crow


def _make_in_maps(x, Wq, bq, Wk, bk, Wv, bv):
    trim, ident, ccol, crow = _host_consts()
    sc = np.float32(1.0 / np.sqrt(DK))
    shared = dict(
        wq=np.ascontiguousarray(Wq, np.float32) * sc,
        bq=np.ascontiguousarray(bq, np.float32) * sc,
        wk=np.ascontiguousarray(Wk, np.float32) * sc,
        bk=np.ascontiguousarray(bk, np.float32) * sc,
        wv=np.ascontiguousarray(Wv, np.float32),
        bv=np.ascontiguousarray(bv, np.float32),
        trim=trim, ident=ident, ccol=ccol, crow=crow,
    )
    return [dict(x=np.ascontiguousarray(x[c], np.float32), **shared)
            for c in range(N_CORES)]


def kernel(x, Wq, bq, Wk, bk, Wv, bv):
    global _RUNNER
    if _RUNNER is None:
        nc = _build_nc(CFG)
        _RUNNER = _make_runner(nc)
    in_maps = _make_in_maps(x, Wq, bq, Wk, bk, Wv, bv)
    res = _RUNNER(in_maps)
    return np.stack([res[c]["out"] for c in range(N_CORES)]).astype(np.float32)

